# revision 9
# baseline (speedup 1.0000x reference)
"""Trainium2 Bass kernel for nn_BaselineMemory (sparse attention memory read + MLP).

Data-parallel over batch: each of 8 NeuronCores handles 256 of 2048 rows.
Pipeline per core:
  x-norm (ACT) -> dist matmul z = x_hat @ y_hat^T - 1 (fp32r, PE)
  -> sparsemax via log-secant threshold iteration (fused ACT relu+bias+accum)
  -> w^T transpose (PE) -> memory read mv^T (bf16, PE)
  -> MLP1 (bf16, transposed layout; b1 fused as per-partition ACT bias + ReLU)
  -> MLP2 (bf16; b2 via rank-1 fp32r matmul) -> fp32 out.
"""
import sys

if "/opt/trn_rl_repo" not in sys.path:
    sys.path.insert(0, "/opt/trn_rl_repo")

import numpy as np
import ml_dtypes

import concourse.bass as bass  # noqa: F401
import concourse.tile as tile
from concourse import bacc, mybir
from concourse.bass_utils import run_bass_kernel_spmd
from concourse.masks import make_identity

P = 128
B_CORE = 256          # batch rows per core
NBT = B_CORE // P     # 2 b-tiles
D = 1024
DC = D // P           # 8 d-chunks
M = 8192
MC512 = M // 512      # 16 m-chunks for dist
MC128 = M // P        # 64 m-chunks for read
H = 2048
HC = H // P           # 16 h-chunks
OUT = 1000
NH = 2                # out halves of 500
N_SECANT = 7          # secant iterations after the 2 init passes

F32 = mybir.dt.float32
F32R = mybir.dt.float32r
BF16 = mybir.dt.bfloat16
AF = mybir.ActivationFunctionType
ALU = mybir.AluOpType
bf16 = ml_dtypes.bfloat16

_EPS = 1e-6


def build():
    nc = bacc.Bacc("TRN2", target_bir_lowering=False, debug=False)

    x_s = nc.dram_tensor("x_s", [NBT, P, D], F32, kind="ExternalInput")
    memT = nc.dram_tensor("memT", [DC, P, M], F32R, kind="ExternalInput")
    mem_bf = nc.dram_tensor("mem_bf", [MC128, P, D], BF16, kind="ExternalInput")
    w1_bf = nc.dram_tensor("w1_bf", [DC, HC, P, P], BF16, kind="ExternalInput")
    w2_bf = nc.dram_tensor("w2_bf", [HC, P, OUT], BF16, kind="ExternalInput")
    b1_t = nc.dram_tensor("b1_t", [P, HC], F32, kind="ExternalInput")
    b2_r = nc.dram_tensor("b2_r", [1, OUT], F32R, kind="ExternalInput")
    out_d = nc.dram_tensor("out", [NBT, P, OUT], F32, kind="ExternalOutput")

    with tile.TileContext(nc) as tc:
        small = tc.alloc_tile_pool(name="small", bufs=1)
        pers = tc.alloc_tile_pool(name="pers", bufs=1)        # xnT (whole kernel)

        ident = small.tile([P, P], F32, tag="ident")
        make_identity(nc, ident[:])
        eps_t = small.tile([P, 1], F32, tag="eps")
        nc.vector.memset(eps_t[:], _EPS)
        b1t = small.tile([P, HC], F32, tag="b1")
        nc.sync.dma_start(b1t[:], b1_t[:])
        b2t = small.tile([1, OUT], F32R, tag="b2")
        nc.sync.dma_start(b2t[:], b2_r[:])
        ones1f = small.tile([1, P], F32, tag="ones1f")
        nc.vector.memset(ones1f[:], 1.0)
        ones1 = small.tile([1, P], F32R, tag="ones1")
        nc.vector.tensor_copy(ones1[:], ones1f[:])

        # ---- x load + normalize + transpose -> xnT [P, dc, 256] fp32r ----
        xnT = pers.tile([P, DC, B_CORE], F32R, tag="xnT")
        xpool = tc.alloc_tile_pool(name="xpool", bufs=1)
        ps_x = tc.alloc_tile_pool(name="ps_x", bufs=2, space="PSUM")
        xn = []
        for bt in range(NBT):
            xt = xpool.tile([P, D], F32, tag=f"x{bt}")
            nc.sync.dma_start(xt[:], x_s[bt])
            ss = small.tile([P, 1], F32, tag=f"ss{bt}")
            sq = xpool.tile([P, D], F32, tag="sqscratch")
            nc.scalar.activation(sq[:], xt[:], AF.Square, accum_out=ss[:])
            nrm = small.tile([P, 1], F32, tag=f"nrm{bt}")
            nc.scalar.activation(nrm[:], ss[:], AF.Sqrt, bias=eps_t[:, 0:1])
            inv = small.tile([P, 1], F32, tag=f"inv{bt}")
            nc.vector.reciprocal(inv[:], nrm[:])
            xnb = xpool.tile([P, D], F32, tag=f"xn{bt}")
            nc.scalar.activation(xnb[:], xt[:], AF.Copy, scale=inv[:, 0:1])
            xn.append(xnb)
        for dc in range(DC):
            pt = ps_x.tile([P, B_CORE], F32, tag="xtr")
            for bt in range(NBT):
                nc.tensor.transpose(
                    pt[:, bt * P:(bt + 1) * P],
                    xn[bt][:, dc * P:(dc + 1) * P], ident[:])
            nc.vector.tensor_copy(xnT[:, dc], pt[:])
        ps_x.release()
        xpool.release()

        # ---- dist matmul: z[bt] [P, M] fp32 (= cos - 1), + chunk maxes of cos ----
        wpool = tc.alloc_tile_pool(name="wpool", bufs=1)
        w = [wpool.tile([P, M], F32, tag=f"w{bt}", name=f"w{bt}") for bt in range(NBT)]
        zpool = tc.alloc_tile_pool(name="zpool", bufs=1)
        z = [zpool.tile([P, M], F32, tag=f"z{bt}", name=f"z{bt}") for bt in range(NBT)]
        mx = [small.tile([P, MC512], F32, tag=f"mx{bt}", name=f"mx{bt}") for bt in range(NBT)]
        mstream = tc.alloc_tile_pool(name="mstream", bufs=2)
        ps_dist = tc.alloc_tile_pool(name="ps_dist", bufs=2, space="PSUM")
        for mc in range(MC512):
            mtile = mstream.tile([P, DC, 512], F32R, tag="memT")
            for dc in range(DC):
                nc.sync.dma_start(mtile[:, dc], memT[dc, :, mc * 512:(mc + 1) * 512])
            for bt in range(NBT):
                zp = ps_dist.tile([P, 512], F32, tag="zp")
                for dc in range(DC):
                    nc.tensor.matmul(
                        zp[:], xnT[:, dc, bt * P:(bt + 1) * P], mtile[:, dc],
                        start=(dc == 0), stop=(dc == DC - 1))
                nc.vector.tensor_scalar_add(
                    z[bt][:, mc * 512:(mc + 1) * 512], zp[:], -1.0)
                nc.vector.reduce_max(
                    mx[bt][:, mc:mc + 1], zp[:], axis=mybir.AxisListType.X)
        ps_dist.release()
        mstream.release()

        # ---- sparsemax thresholds via log-secant; final pass leaves w fp32 ----
        CAP_OFF = 1e-4

        def s_pass(bt, negtau, s_out):
            nc.scalar.activation(
                w[bt][:], z[bt][:], AF.Relu,
                bias=negtau[:, 0:1], accum_out=s_out[:])

        st = {}
        for bt in range(NBT):
            rm = small.tile([P, 1], F32, tag=f"rm{bt}")
            nc.vector.reduce_max(rm[:], mx[bt][:], axis=mybir.AxisListType.X)
            nc.vector.tensor_scalar_add(rm[:], rm[:], -1.0)  # rowmax of z
            cap = small.tile([P, 1], F32, tag=f"cap{bt}")
            nc.vector.tensor_scalar_add(cap[:], rm[:], -CAP_OFF)
            tau_p = small.tile([P, 1], F32, tag=f"taup{bt}")
            nc.vector.tensor_scalar_add(tau_p[:], rm[:], -1.0)
            ntau = small.tile([P, 1], F32, tag=f"ntau{bt}")
            nc.vector.tensor_scalar_mul(ntau[:], tau_p[:], -1.0)
            s_v = small.tile([P, 1], F32, tag=f"s{bt}")
            s_pass(bt, ntau, s_v)
            l_p = small.tile([P, 1], F32, tag=f"lp{bt}")
            nc.scalar.activation(l_p[:], s_v[:], AF.Ln)
            tau_c = small.tile([P, 1], F32, tag=f"tauc{bt}")
            nc.vector.tensor_scalar(
                out=tau_c[:], in0=s_v[:], scalar1=-1.0, scalar2=1.0 / M,
                op0=ALU.add, op1=ALU.mult)
            nc.vector.tensor_add(tau_c[:], tau_c[:], tau_p[:])
            nc.vector.tensor_tensor(tau_c[:], tau_c[:], cap[:], ALU.min)
            nc.vector.tensor_scalar_mul(ntau[:], tau_c[:], -1.0)
            s_pass(bt, ntau, s_v)
            l_c = small.tile([P, 1], F32, tag=f"lc{bt}")
            nc.scalar.activation(l_c[:], s_v[:], AF.Ln)
            st[bt] = (cap, tau_p, tau_c, l_p, l_c, ntau, s_v)

        for it in range(N_SECANT):
            for bt in range(NBT):
                cap, tau_p, tau_c, l_p, l_c, ntau, s_v = st[bt]
                dl = small.tile([P, 1], F32, tag=f"dl{bt}")
                nc.vector.tensor_sub(dl[:], l_p[:], l_c[:])
                # adl = max(|dl|, 1e-12); q = min(|dt|/adl, 2); step = l_c*q
                neg = small.tile([P, 1], F32, tag=f"neg{bt}")
                nc.vector.tensor_scalar_mul(neg[:], dl[:], -1.0)
                nc.vector.tensor_max(dl[:], dl[:], neg[:])
                nc.vector.tensor_scalar_max(dl[:], dl[:], 1e-12)
                rdl = small.tile([P, 1], F32, tag=f"rdl{bt}")
                nc.vector.reciprocal(rdl[:], dl[:])
                dt = small.tile([P, 1], F32, tag=f"dt{bt}")
                nc.vector.tensor_sub(dt[:], tau_c[:], tau_p[:])
                nc.vector.tensor_scalar_mul(neg[:], dt[:], -1.0)
                nc.vector.tensor_max(dt[:], dt[:], neg[:])
                step = small.tile([P, 1], F32, tag=f"step{bt}")
                nc.vector.tensor_mul(step[:], dt[:], rdl[:])
                nc.vector.tensor_scalar_min(step[:], step[:], 2.0)
                nc.vector.tensor_mul(step[:], step[:], l_c[:])
                nc.vector.tensor_copy(tau_p[:], tau_c[:])
                nc.vector.tensor_add(tau_c[:], tau_c[:], step[:])
                nc.vector.tensor_tensor(tau_c[:], tau_c[:], cap[:], ALU.min)
                nc.vector.tensor_copy(l_p[:], l_c[:])
                nc.vector.tensor_scalar_mul(ntau[:], tau_c[:], -1.0)
                s_pass(bt, ntau, s_v)  # last iteration: w = relu(z - tau_final)
                if it != N_SECANT - 1:
                    nc.scalar.activation(l_c[:], s_v[:], AF.Ln)
        zpool.release()

        # ---- w^T transposes (all upfront), then read matmul with 8 psum banks ----
        late = tc.alloc_tile_pool(name="late", bufs=1)
        wTt = late.tile([P, MC128, B_CORE], BF16, tag="wT")
        ps_trw = tc.alloc_tile_pool(name="ps_trw", bufs=2, space="PSUM")
        for mc in range(MC128):
            tp = ps_trw.tile([P, B_CORE], F32, tag="wtr")
            for bt in range(NBT):
                nc.tensor.transpose(
                    tp[:, bt * P:(bt + 1) * P],
                    w[bt][:, mc * P:(mc + 1) * P], ident[:])
            nc.vector.tensor_copy(wTt[:, mc], tp[:])
        ps_trw.release()

        mbstream = tc.alloc_tile_pool(name="mbstream", bufs=3)
        ps_mv = tc.alloc_tile_pool(name="ps_mv", bufs=1, space="PSUM")
        mv_ps = [ps_mv.tile([P, B_CORE], F32, tag=f"mv{dc}", name=f"mv{dc}")
                 for dc in range(DC)]
        for mc in range(MC128):
            mtile = mbstream.tile([P, D], BF16, tag="membf")
            nc.sync.dma_start(mtile[:], mem_bf[mc])
            for dc in range(DC):
                nc.tensor.matmul(
                    mv_ps[dc][:],
                    mtile[:, dc * P:(dc + 1) * P], wTt[:, mc],
                    start=(mc == 0), stop=(mc == MC128 - 1))
        mvT = late.tile([P, DC, B_CORE], BF16, tag="mvT")
        for dc in range(DC):
            nc.scalar.copy(mvT[:, dc], mv_ps[dc][:])
        ps_mv.release()
        mbstream.release()

        # ---- MLP1: hT[hc] = relu(sum_dc W1-block^T @ mvT[dc] + b1[hc]) ----
        w1t = late.tile([P, DC, HC, P], BF16, tag="w1")
        for dc in range(DC):
            for hc in range(HC):
                nc.sync.dma_start(w1t[:, dc, hc], w1_bf[dc, hc])
        hT = late.tile([P, HC, B_CORE], BF16, tag="hT")
        ps_h = tc.alloc_tile_pool(name="ps_h", bufs=2, space="PSUM")
        for hc in range(HC):
            hp = ps_h.tile([P, B_CORE], F32, tag="hp")
            for dc in range(DC):
                nc.tensor.matmul(
                    hp[:], w1t[:, dc, hc], mvT[:, dc],
                    start=(dc == 0), stop=(dc == DC - 1))
            nc.scalar.activation(
                hT[:, hc], hp[:], AF.Relu, bias=b1t[:, hc:hc + 1])
        ps_h.release()

        # ---- MLP2: out[bt] = hT-blocks^T @ W2 + b2 ----
        ps_o = tc.alloc_tile_pool(name="ps_o", bufs=2, space="PSUM")
        w2pool = tc.alloc_tile_pool(name="w2s", bufs=4)
        for bt in range(NBT):
            osb = late.tile([P, OUT], F32, tag=f"osb{bt}")
            for nh in range(NH):
                NW = OUT // NH
                op = ps_o.tile([P, NW], F32, tag="op")
                for kc in range(HC):
                    w2c = w2pool.tile([P, NW], BF16, tag="w2c")
                    nc.sync.dma_start(
                        w2c[:], w2_bf[kc, :, nh * NW:(nh + 1) * NW])
                    nc.tensor.matmul(
                        op[:], hT[:, kc, bt * P:(bt + 1) * P], w2c[:],
                        start=(kc == 0), stop=False)
                nc.tensor.matmul(
                    op[:], ones1[:], b2t[:, nh * NW:(nh + 1) * NW],
                    start=False, stop=True)
                nc.scalar.copy(osb[:, nh * NW:(nh + 1) * NW], op[:])
            nc.sync.dma_start(out_d[bt], osb[:])
        w2pool.release()
        ps_o.release()
        late.release()
        wpool.release()
        pers.release()
        small.release()

    nc.compile()
    return nc


_CACHED = None


def _prep(inputs):
    x = np.ascontiguousarray(inputs["encoder_output"], dtype=np.float32)
    mem = np.ascontiguousarray(inputs["memory_set"], dtype=np.float32)
    W1 = np.ascontiguousarray(inputs["W1"], dtype=np.float32)
    b1 = np.ascontiguousarray(inputs["b1"], dtype=np.float32)
    W2 = np.ascontiguousarray(inputs["W2"], dtype=np.float32)
    b2 = np.ascontiguousarray(inputs["b2"], dtype=np.float32)

    inv_ny = 1.0 / np.sqrt((mem * mem).sum(1) + _EPS)
    memT_hat = np.ascontiguousarray(
        (mem.T * inv_ny[None, :]).astype(np.float32).reshape(DC, P, M))
    mem_bfv = np.ascontiguousarray(mem.astype(bf16).reshape(MC128, P, D))
    w1_blk = np.ascontiguousarray(
        W1.astype(bf16).reshape(DC, P, HC, P).transpose(0, 2, 1, 3))
    w2_blk = np.ascontiguousarray(W2.astype(bf16).reshape(HC, P, OUT))
    b1_tiles = np.ascontiguousarray(b1.reshape(HC, P).T.astype(np.float32))
    b2_row = np.ascontiguousarray(b2.reshape(1, OUT).astype(np.float32))

    shared = {
        "memT": memT_hat, "mem_bf": mem_bfv, "w1_bf": w1_blk,
        "w2_bf": w2_blk, "b1_t": b1_tiles, "b2_r": b2_row,
    }
    in_maps = []
    for c in range(8):
        xs = np.ascontiguousarray(
            x[c * B_CORE:(c + 1) * B_CORE].reshape(NBT, P, D))
        in_maps.append({"x_s": xs, **shared})
    return in_maps


def kernel(**inputs) -> np.ndarray:
    global _CACHED
    if _CACHED is None:
        _CACHED = build()
    nc = _CACHED
    in_maps = _prep(inputs)
    res = run_bass_kernel_spmd(nc, in_maps, core_ids=list(range(8)))
    return np.concatenate(
        [r["out"].reshape(B_CORE, OUT) for r in res.results], axis=0)


# revision 11
# speedup vs baseline: 1.1962x; 1.1962x over previous
"""Trainium2 Bass kernel for nn_BaselineMemory (sparse attention memory read + MLP).

Data-parallel over batch: each of 8 NeuronCores handles 256 of 2048 rows.
Pipeline per core:
  x-norm (ACT) -> dist matmul z = x_hat @ y_hat^T - 1 (fp32r, PE)
  -> sparsemax via log-secant threshold iteration (ACT relu+bias+accum on the
     head of m, DVE chunked max+sum on the tail: S = sum max(z,tau) - n*tau)
  -> w^T transpose (PE) -> memory read mv^T (bf16, PE)
  -> MLP1 (bf16, transposed layout; b1 fused as per-partition ACT bias + ReLU)
  -> MLP2 (bf16; b2 via rank-1 fp32r matmul) -> fp32 out.
"""
import sys

if "/opt/trn_rl_repo" not in sys.path:
    sys.path.insert(0, "/opt/trn_rl_repo")

import numpy as np
import ml_dtypes

import concourse.bass as bass  # noqa: F401
import concourse.tile as tile
from concourse import bacc, mybir
from concourse.bass_utils import run_bass_kernel_spmd
from concourse.masks import make_identity

P = 128
B_CORE = 256          # batch rows per core
NBT = B_CORE // P     # 2 b-tiles
D = 1024
DC = D // P           # 8 d-chunks
M = 8192
MC512 = M // 512      # 16 m-chunks for dist
MC128 = M // P        # 64 m-chunks for read
H = 2048
HC = H // P           # 16 h-chunks
OUT = 1000
NH = 2                # out halves of 500
N_SECANT = 7          # secant iterations after the init pass
MA = 4608             # ACT handles m [0, MA); DVE chunks handle [MA, M)
NDV = (M - MA) // 512  # 7 DVE chunks of 512

F32 = mybir.dt.float32
F32R = mybir.dt.float32r
BF16 = mybir.dt.bfloat16
AF = mybir.ActivationFunctionType
ALU = mybir.AluOpType
AX = mybir.AxisListType
bf16 = ml_dtypes.bfloat16

_EPS = 1e-6


def build():
    nc = bacc.Bacc("TRN2", target_bir_lowering=False, debug=False)

    x_s = nc.dram_tensor("x_s", [NBT, P, D], F32, kind="ExternalInput")
    memT = nc.dram_tensor("memT", [DC, P, M], F32R, kind="ExternalInput")
    mem_bf = nc.dram_tensor("mem_bf", [MC128, P, D], BF16, kind="ExternalInput")
    # host-prepped partition-major layouts (contiguous per-partition runs)
    w1_bf = nc.dram_tensor("w1_bf", [P, DC, HC, P], BF16, kind="ExternalInput")
    w2_bf = nc.dram_tensor("w2_bf", [P, HC, OUT], BF16, kind="ExternalInput")
    b1_t = nc.dram_tensor("b1_t", [P, HC], F32, kind="ExternalInput")
    b2_r = nc.dram_tensor("b2_r", [1, OUT], F32R, kind="ExternalInput")
    out_d = nc.dram_tensor("out", [NBT, P, OUT], F32, kind="ExternalOutput")

    with tile.TileContext(nc) as tc:
        small = tc.alloc_tile_pool(name="small", bufs=1)
        pers = tc.alloc_tile_pool(name="pers", bufs=1)

        ident = small.tile([P, P], F32, tag="ident")
        make_identity(nc, ident[:])
        eps_t = small.tile([P, 1], F32, tag="eps")
        nc.vector.memset(eps_t[:], _EPS)
        b1t = small.tile([P, HC], F32, tag="b1")
        nc.sync.dma_start(b1t[:], b1_t[:])
        b2t = small.tile([1, OUT], F32R, tag="b2")
        nc.sync.dma_start(b2t[:], b2_r[:])
        ones1f = small.tile([1, P], F32, tag="ones1f")
        nc.vector.memset(ones1f[:], 1.0)
        ones1 = small.tile([1, P], F32R, tag="ones1")
        nc.vector.tensor_copy(ones1[:], ones1f[:])

        # ---- x load + normalize + transpose -> xnT [P, dc, 256] fp32r ----
        xnT = pers.tile([P, DC, B_CORE], F32R, tag="xnT")
        xpool = tc.alloc_tile_pool(name="xpool", bufs=1)
        ps_x = tc.alloc_tile_pool(name="ps_x", bufs=2, space="PSUM")
        xn = []
        for bt in range(NBT):
            xt = xpool.tile([P, D], F32, tag=f"x{bt}")
            nc.sync.dma_start(xt[:], x_s[bt])
            ss = small.tile([P, 1], F32, tag=f"ss{bt}")
            sq = xpool.tile([P, D], F32, tag="sqscratch")
            nc.scalar.activation(sq[:], xt[:], AF.Square, accum_out=ss[:])
            nrm = small.tile([P, 1], F32, tag=f"nrm{bt}")
            nc.scalar.activation(nrm[:], ss[:], AF.Sqrt, bias=eps_t[:, 0:1])
            inv = small.tile([P, 1], F32, tag=f"inv{bt}")
            nc.vector.reciprocal(inv[:], nrm[:])
            xnb = xpool.tile([P, D], F32, tag=f"xn{bt}")
            nc.scalar.activation(xnb[:], xt[:], AF.Copy, scale=inv[:, 0:1])
            xn.append(xnb)
        for dc in range(DC):
            pt = ps_x.tile([P, B_CORE], F32, tag="xtr")
            for bt in range(NBT):
                nc.tensor.transpose(
                    pt[:, bt * P:(bt + 1) * P],
                    xn[bt][:, dc * P:(dc + 1) * P], ident[:])
            nc.vector.tensor_copy(xnT[:, dc], pt[:])
        ps_x.release()
        xpool.release()

        # Slot-sharing pools: wpool tags w0/w1 (32KB slots), zpool tags z0/z1.
        wpool = tc.alloc_tile_pool(name="wpool", bufs=1)
        w = [wpool.tile([P, M], F32, tag=f"w{bt}", name=f"w{bt}") for bt in range(NBT)]
        zpool = tc.alloc_tile_pool(name="zpool", bufs=1)
        z = [zpool.tile([P, M], F32, tag=f"z{bt}", name=f"z{bt}") for bt in range(NBT)]
        mstream = tc.alloc_tile_pool(name="mstream", bufs=2)

        # ---- dist matmul: z[bt] [P, M] fp32 (= cos - 1) + chunk maxes + sums ----
        mx = [small.tile([P, MC512], F32, tag=f"mx{bt}", name=f"mx{bt}")
              for bt in range(NBT)]
        zsum = [small.tile([P, MC512], F32, tag=f"zs{bt}", name=f"zs{bt}")
                for bt in range(NBT)]
        ps_dist = tc.alloc_tile_pool(name="ps_dist", bufs=4, space="PSUM")
        for mc in range(MC512):
            mtile = mstream.tile([P, DC, 512], F32R, tag="memT")
            nc.sync.dma_start(
                mtile[:], memT[:, :, mc * 512:(mc + 1) * 512]
                .rearrange("d p m -> p d m"))
            for bt in range(NBT):
                zp = ps_dist.tile([P, 512], F32, tag="zp")
                for dc in range(DC):
                    nc.tensor.matmul(
                        zp[:], xnT[:, dc, bt * P:(bt + 1) * P], mtile[:, dc],
                        start=(dc == 0), stop=(dc == DC - 1))
                nc.vector.tensor_scalar(
                    out=z[bt][:, mc * 512:(mc + 1) * 512], in0=zp[:],
                    scalar1=-1.0, scalar2=None, op0=ALU.add, op1=ALU.add,
                    accum_out=zsum[bt][:, mc:mc + 1])
                nc.vector.reduce_max(
                    mx[bt][:, mc:mc + 1], zp[:], axis=AX.X)
        ps_dist.release()

        # ---- sparsemax via log-secant; S(tau) = ACT head + DVE tail chunks ----
        ps_warm = tc.alloc_tile_pool(name="ps_warm", bufs=2, space="PSUM")
        CAP_OFF = 1e-4

        tail_scr = small.tile([P, 512], F32, tag="tailscr")

        def s_pass(bt, stt):
            tau_c, ntau = stt["tau_c"], stt["ntau"]
            s_act, gacc, s_v = stt["s_act"], stt["gacc"], stt["s_v"]
            nc.scalar.activation(
                w[bt][:, 0:MA], z[bt][:, 0:MA], AF.Relu,
                bias=ntau[:, 0:1], accum_out=s_act[:])
            for c in range(NDV):
                off = MA + c * 512
                # relu in two DVE ops: max into scratch, then (x - tau) with
                # fused add-reduce accum (sums small positives -> no cancellation)
                nc.vector.tensor_scalar(
                    out=tail_scr[:], in0=z[bt][:, off:off + 512],
                    scalar1=tau_c[:, 0:1], scalar2=None, op0=ALU.max)
                nc.vector.tensor_scalar(
                    out=w[bt][:, off:off + 512], in0=tail_scr[:],
                    scalar1=tau_c[:, 0:1], scalar2=None,
                    op0=ALU.subtract, op1=ALU.add, accum_out=gacc[:, c:c + 1])
            # S = s_act + sum(gacc)
            gs = stt["gs"]
            nc.vector.reduce_sum(gs[:], gacc[:], axis=AX.X)
            nc.vector.tensor_add(s_v[:], gs[:], s_act[:])
            # PE warmers: keep HAM at 8/8 through the sparsemax window
            wp = ps_warm.tile([P, 512], F32, tag="warm")
            nc.tensor.matmul(wp[:], ident[:], w[bt][:, 0:512],
                             start=True, stop=True)
            nc.tensor.matmul(wp[:], ident[:], w[bt][:, 512:1024],
                             start=True, stop=True)

        st = {}
        for bt in range(NBT):
            stt = {}
            for nm in ["rm", "cap", "tau_p", "tau_c", "l_p", "l_c", "ntau",
                       "s_v", "s_act", "gs", "corr", "dl", "rdl", "dt",
                       "step", "neg"]:
                stt[nm] = small.tile([P, 1], F32, tag=f"{nm}{bt}", name=f"{nm}{bt}")
            stt["gacc"] = small.tile([P, NDV], F32, tag=f"gacc{bt}", name=f"gacc{bt}")
            st[bt] = stt
            rm, cap, tau_p, tau_c = stt["rm"], stt["cap"], stt["tau_p"], stt["tau_c"]
            l_p, ntau, s_v = stt["l_p"], stt["ntau"], stt["s_v"]
            nc.vector.reduce_max(rm[:], mx[bt][:], axis=AX.X)
            nc.vector.tensor_scalar_add(rm[:], rm[:], -1.0)  # rowmax of z
            nc.vector.tensor_scalar_add(cap[:], rm[:], -CAP_OFF)
            nc.vector.tensor_scalar_add(tau_p[:], rm[:], -1.0)
            # analytic S0 = sum(z) - M*tau_p (tau_p = rowmax-1; <= true S, safe)
            zs = stt["gs"]
            nc.vector.reduce_sum(zs[:], zsum[bt][:], axis=AX.X)
            nc.vector.tensor_scalar_mul(s_v[:], tau_p[:], -float(M))
            nc.vector.tensor_add(s_v[:], s_v[:], zs[:])
            nc.vector.tensor_scalar_max(s_v[:], s_v[:], 1.0)  # guard ln<=0
            nc.scalar.activation(l_p[:], s_v[:], AF.Ln)
            # tau_c = tau_p + (S0-1)/M, capped
            nc.vector.tensor_scalar(
                out=tau_c[:], in0=s_v[:], scalar1=-1.0, scalar2=1.0 / M,
                op0=ALU.add, op1=ALU.mult)
            nc.vector.tensor_add(tau_c[:], tau_c[:], tau_p[:])
            nc.vector.tensor_tensor(tau_c[:], tau_c[:], cap[:], ALU.min)
            nc.vector.tensor_scalar_mul(ntau[:], tau_c[:], -1.0)
            s_pass(bt, stt)
            nc.scalar.activation(stt["l_c"][:], s_v[:], AF.Ln)

        for it in range(N_SECANT):
            for bt in range(NBT):
                stt = st[bt]
                cap, tau_p, tau_c = stt["cap"], stt["tau_p"], stt["tau_c"]
                l_p, l_c, ntau, s_v = stt["l_p"], stt["l_c"], stt["ntau"], stt["s_v"]
                dl, rdl, dt = stt["dl"], stt["rdl"], stt["dt"]
                step, neg = stt["step"], stt["neg"]
                nc.vector.tensor_sub(dl[:], l_p[:], l_c[:])
                nc.vector.tensor_scalar_mul(neg[:], dl[:], -1.0)
                nc.vector.tensor_max(dl[:], dl[:], neg[:])
                nc.vector.tensor_scalar_max(dl[:], dl[:], 1e-12)
                nc.vector.reciprocal(rdl[:], dl[:])
                nc.vector.tensor_sub(dt[:], tau_c[:], tau_p[:])
                nc.vector.tensor_scalar_mul(neg[:], dt[:], -1.0)
                nc.vector.tensor_max(dt[:], dt[:], neg[:])
                nc.vector.tensor_mul(step[:], dt[:], rdl[:])
                nc.vector.tensor_scalar_min(step[:], step[:], 2.0)
                nc.vector.tensor_mul(step[:], step[:], l_c[:])
                nc.vector.tensor_copy(tau_p[:], tau_c[:])
                nc.vector.tensor_add(tau_c[:], tau_c[:], step[:])
                nc.vector.tensor_tensor(tau_c[:], tau_c[:], cap[:], ALU.min)
                nc.vector.tensor_copy(l_p[:], l_c[:])
                nc.vector.tensor_scalar_mul(ntau[:], tau_c[:], -1.0)
                s_pass(bt, stt)  # last iteration leaves w (head exact relu)
                if it != N_SECANT - 1:
                    nc.scalar.activation(l_c[:], s_v[:], AF.Ln)
        ps_warm.release()

        # ---- w^T transposes (wT reuses z0's slot) ----
        wTt = zpool.tile([P, MC128, B_CORE], BF16, tag="z0", name="wTt")
        ps_trw = tc.alloc_tile_pool(name="ps_trw", bufs=4, space="PSUM")
        for mc in range(MC128):
            tp = ps_trw.tile([P, B_CORE], F32, tag="wtr")
            for bt in range(NBT):
                nc.tensor.transpose(
                    tp[:, bt * P:(bt + 1) * P],
                    w[bt][:, mc * P:(mc + 1) * P], ident[:])
            nc.vector.tensor_copy(wTt[:, mc], tp[:])
        ps_trw.release()

        # W1 blocks prefetch into z1's slot (one contiguous DMA)
        w1t = zpool.tile([P, DC, HC, P], BF16, tag="z1", name="w1t")
        nc.sync.dma_start(w1t[:], w1_bf[:])

        # ---- read matmul: mvT[dc] accumulated in its own psum bank ----
        ps_mv = tc.alloc_tile_pool(name="ps_mv", bufs=1, space="PSUM")
        mv_ps = [ps_mv.tile([P, B_CORE], F32, tag=f"mv{dc}", name=f"mv{dc}")
                 for dc in range(DC)]
        for mc4 in range(MC128 // 4):
            mtile = mstream.tile([P, 4, D], BF16, tag="memT", name="membf")
            nc.sync.dma_start(
                mtile[:], mem_bf[mc4 * 4:(mc4 + 1) * 4].rearrange("c p d -> p c d"))
            for c in range(4):
                mc = mc4 * 4 + c
                for dc in range(DC):
                    nc.tensor.matmul(
                        mv_ps[dc][:],
                        mtile[:, c, dc * P:(dc + 1) * P], wTt[:, mc],
                        start=(mc == 0), stop=(mc == MC128 - 1))

        mvT = wpool.tile([P, DC, B_CORE], BF16, tag="w0", name="mvT")
        for dc in range(DC):
            nc.scalar.copy(mvT[:, dc], mv_ps[dc][:])
        ps_mv.release()

        # ---- MLP1: hT[hc] = relu(sum_dc W1-block^T @ mvT[dc] + b1[hc]) ----
        hT = wpool.tile([P, HC, B_CORE], BF16, tag="w1", name="hT")
        ps_h = tc.alloc_tile_pool(name="ps_h", bufs=4, space="PSUM")
        for hc in range(HC):
            hp = ps_h.tile([P, B_CORE], F32, tag="hp")
            for dc in range(DC):
                nc.tensor.matmul(
                    hp[:], w1t[:, dc, hc], mvT[:, dc],
                    start=(dc == 0), stop=(dc == DC - 1))
            nc.scalar.activation(
                hT[:, hc], hp[:], AF.Relu, bias=b1t[:, hc:hc + 1])
        ps_h.release()

        # ---- MLP2: out[bt] = hT-blocks^T @ W2 + b2 (nh outer, W2 slab DMA) ----
        ps_o = tc.alloc_tile_pool(name="ps_o", bufs=4, space="PSUM")
        osb = [small.tile([P, OUT], F32, tag=f"osb{bt}", name=f"osb{bt}")
               for bt in range(NBT)]
        NW = OUT // NH
        for nh in range(NH):
            w2slab = mstream.tile([P, HC, NW], BF16, tag="memT", name="w2slab")
            nc.sync.dma_start(w2slab[:], w2_bf[:, :, nh * NW:(nh + 1) * NW])
            ops = [ps_o.tile([P, NW], F32, tag=f"op{bt}", name=f"op{bt}")
                   for bt in range(NBT)]
            for kc in range(HC):
                for bt in range(NBT):
                    nc.tensor.matmul(
                        ops[bt][:], hT[:, kc, bt * P:(bt + 1) * P],
                        w2slab[:, kc], start=(kc == 0), stop=False)
            for bt in range(NBT):
                nc.tensor.matmul(
                    ops[bt][:], ones1[:], b2t[:, nh * NW:(nh + 1) * NW],
                    start=False, stop=True)
                nc.scalar.copy(osb[bt][:, nh * NW:(nh + 1) * NW], ops[bt][:])
        for bt in range(NBT):
            nc.sync.dma_start(out_d[bt], osb[bt][:])
        ps_o.release()

        mstream.release()
        zpool.release()
        wpool.release()
        pers.release()
        small.release()

    nc.compile()
    return nc


_CACHED = None


def _prep(inputs):
    x = np.ascontiguousarray(inputs["encoder_output"], dtype=np.float32)
    mem = np.ascontiguousarray(inputs["memory_set"], dtype=np.float32)
    W1 = np.ascontiguousarray(inputs["W1"], dtype=np.float32)
    b1 = np.ascontiguousarray(inputs["b1"], dtype=np.float32)
    W2 = np.ascontiguousarray(inputs["W2"], dtype=np.float32)
    b2 = np.ascontiguousarray(inputs["b2"], dtype=np.float32)

    inv_ny = 1.0 / np.sqrt((mem * mem).sum(1) + _EPS)
    memT_hat = np.ascontiguousarray(
        (mem.T * inv_ny[None, :]).astype(np.float32).reshape(DC, P, M))
    mem_bfv = np.ascontiguousarray(mem.astype(bf16).reshape(MC128, P, D))
    # partition-major blocks: w1[p, dc, hc, c] = W1[dc*128+p, hc*128+c]
    w1_blk = np.ascontiguousarray(
        W1.astype(bf16).reshape(DC, P, HC, P).transpose(1, 0, 2, 3))
    # w2[p, kc, o] = W2[kc*128+p, o]
    w2_blk = np.ascontiguousarray(
        W2.astype(bf16).reshape(HC, P, OUT).transpose(1, 0, 2))
    b1_tiles = np.ascontiguousarray(b1.reshape(HC, P).T.astype(np.float32))
    b2_row = np.ascontiguousarray(b2.reshape(1, OUT).astype(np.float32))

    shared = {
        "memT": memT_hat, "mem_bf": mem_bfv, "w1_bf": w1_blk,
        "w2_bf": w2_blk, "b1_t": b1_tiles, "b2_r": b2_row,
    }
    in_maps = []
    for c in range(8):
        xs = np.ascontiguousarray(
            x[c * B_CORE:(c + 1) * B_CORE].reshape(NBT, P, D))
        in_maps.append({"x_s": xs, **shared})
    return in_maps


def kernel(**inputs) -> np.ndarray:
    global _CACHED
    if _CACHED is None:
        _CACHED = build()
    nc = _CACHED
    in_maps = _prep(inputs)
    res = run_bass_kernel_spmd(nc, in_maps, core_ids=list(range(8)))
    return np.concatenate(
        [r["out"].reshape(B_CORE, OUT) for r in res.results], axis=0)


# revision 12
# speedup vs baseline: 1.5605x; 1.3046x over previous
"""Trainium2 Bass kernel for nn_BaselineMemory (sparse attention memory read + MLP).

Data-parallel over batch: each of 8 NeuronCores handles 256 of 2048 rows.
Pipeline per core:
  x-norm (ACT) -> dist matmul z = x_hat @ y_hat^T - 1 (fp32r, PE)
  -> sparsemax via log-secant threshold iteration (ACT relu+bias+accum on the
     head of m, DVE chunked max+sum on the tail: S = sum max(z,tau) - n*tau)
  -> w^T transpose (PE) -> memory read mv^T (bf16, PE)
  -> MLP1 (bf16, transposed layout; b1 fused as per-partition ACT bias + ReLU)
  -> MLP2 (bf16; b2 via rank-1 fp32r matmul) -> fp32 out.
"""
import sys

if "/opt/trn_rl_repo" not in sys.path:
    sys.path.insert(0, "/opt/trn_rl_repo")

import numpy as np
import ml_dtypes

import concourse.bass as bass  # noqa: F401
import concourse.tile as tile
from concourse import bacc, mybir
from concourse.bass_utils import run_bass_kernel_spmd
from concourse.masks import make_identity

P = 128
B_CORE = 256          # batch rows per core
NBT = B_CORE // P     # 2 b-tiles
D = 1024
DC = D // P           # 8 d-chunks
M = 8192
MC512 = M // 512      # 16 m-chunks for dist
MC128 = M // P        # 64 m-chunks for read
H = 2048
HC = H // P           # 16 h-chunks
OUT = 1000
NH = 2                # out halves of 500
N_SECANT = 6          # secant iterations after the init pass
MA = 6144             # ACT handles m [0, MA); DVE chunks handle [MA, M)
NDV = (M - MA) // 512  # 7 DVE chunks of 512

F32 = mybir.dt.float32
F32R = mybir.dt.float32r
BF16 = mybir.dt.bfloat16
AF = mybir.ActivationFunctionType
ALU = mybir.AluOpType
AX = mybir.AxisListType
bf16 = ml_dtypes.bfloat16

_EPS = 1e-6


def build():
    nc = bacc.Bacc("TRN2", target_bir_lowering=False, debug=False)

    x_s = nc.dram_tensor("x_s", [NBT, P, D], F32, kind="ExternalInput")
    memT = nc.dram_tensor("memT", [DC, P, M], F32R, kind="ExternalInput")
    mem_bf = nc.dram_tensor("mem_bf", [MC128, P, D], BF16, kind="ExternalInput")
    # host-prepped partition-major layouts (contiguous per-partition runs)
    w1_bf = nc.dram_tensor("w1_bf", [P, DC, HC, P], BF16, kind="ExternalInput")
    w2_bf = nc.dram_tensor("w2_bf", [P, HC, OUT], BF16, kind="ExternalInput")
    b1_t = nc.dram_tensor("b1_t", [P, HC], F32, kind="ExternalInput")
    b2_r = nc.dram_tensor("b2_r", [1, OUT], F32R, kind="ExternalInput")
    out_d = nc.dram_tensor("out", [NBT, P, OUT], F32, kind="ExternalOutput")

    with tile.TileContext(nc) as tc:
        small = tc.alloc_tile_pool(name="small", bufs=1)
        pers = tc.alloc_tile_pool(name="pers", bufs=1)

        ident = small.tile([P, P], F32, tag="ident")
        make_identity(nc, ident[:])
        eps_t = small.tile([P, 1], F32, tag="eps")
        nc.vector.memset(eps_t[:], _EPS)
        b1t = small.tile([P, HC], F32, tag="b1")
        nc.sync.dma_start(b1t[:], b1_t[:])
        b2t = small.tile([1, OUT], F32R, tag="b2")
        nc.sync.dma_start(b2t[:], b2_r[:])
        ones1f = small.tile([1, P], F32, tag="ones1f")
        nc.vector.memset(ones1f[:], 1.0)
        ones1 = small.tile([1, P], F32R, tag="ones1")
        nc.vector.tensor_copy(ones1[:], ones1f[:])

        # ---- x load + normalize + transpose -> xnT [P, dc, 256] fp32r ----
        xnT = pers.tile([P, DC, B_CORE], F32R, tag="xnT")
        xpool = tc.alloc_tile_pool(name="xpool", bufs=1)
        ps_x = tc.alloc_tile_pool(name="ps_x", bufs=2, space="PSUM")
        xn = []
        for bt in range(NBT):
            xt = xpool.tile([P, D], F32, tag=f"x{bt}")
            nc.sync.dma_start(xt[:], x_s[bt])
            ss = small.tile([P, 1], F32, tag=f"ss{bt}")
            sq = xpool.tile([P, D], F32, tag="sqscratch")
            nc.scalar.activation(sq[:], xt[:], AF.Square, accum_out=ss[:])
            nrm = small.tile([P, 1], F32, tag=f"nrm{bt}")
            nc.scalar.activation(nrm[:], ss[:], AF.Sqrt, bias=eps_t[:, 0:1])
            inv = small.tile([P, 1], F32, tag=f"inv{bt}")
            nc.vector.reciprocal(inv[:], nrm[:])
            xnb = xpool.tile([P, D], F32, tag=f"xn{bt}")
            nc.scalar.activation(xnb[:], xt[:], AF.Copy, scale=inv[:, 0:1])
            xn.append(xnb)
        for dc in range(DC):
            pt = ps_x.tile([P, B_CORE], F32, tag="xtr")
            for bt in range(NBT):
                nc.tensor.transpose(
                    pt[:, bt * P:(bt + 1) * P],
                    xn[bt][:, dc * P:(dc + 1) * P], ident[:])
            nc.vector.tensor_copy(xnT[:, dc], pt[:])
        ps_x.release()
        xpool.release()

        # Slot-sharing pools: wpool tags w0/w1 (32KB slots), zpool tags z0/z1.
        wpool = tc.alloc_tile_pool(name="wpool", bufs=1)
        w = [wpool.tile([P, M], F32, tag=f"w{bt}", name=f"w{bt}") for bt in range(NBT)]
        zpool = tc.alloc_tile_pool(name="zpool", bufs=1)
        z = [zpool.tile([P, M], F32, tag=f"z{bt}", name=f"z{bt}") for bt in range(NBT)]
        mstream = tc.alloc_tile_pool(name="mstream", bufs=2)

        # ---- dist matmul: z[bt] [P, M] fp32 (= cos - 1) + chunk maxes + sums ----
        mx = [small.tile([P, MC512], F32, tag=f"mx{bt}", name=f"mx{bt}")
              for bt in range(NBT)]
        zsum = [small.tile([P, MC512], F32, tag=f"zs{bt}", name=f"zs{bt}")
                for bt in range(NBT)]
        ps_dist = tc.alloc_tile_pool(name="ps_dist", bufs=4, space="PSUM")
        for mc in range(MC512):
            mtile = mstream.tile([P, DC, 512], F32R, tag="memT")
            for dq in range(4):
                nc.sync.dma_start(
                    mtile[:, dq * 2:(dq + 1) * 2],
                    memT[dq * 2:(dq + 1) * 2, :, mc * 512:(mc + 1) * 512]
                    .rearrange("d p m -> p d m"))
            for bt in range(NBT):
                zp = ps_dist.tile([P, 512], F32, tag="zp")
                for dc in range(DC):
                    nc.tensor.matmul(
                        zp[:], xnT[:, dc, bt * P:(bt + 1) * P], mtile[:, dc],
                        start=(dc == 0), stop=(dc == DC - 1))
                nc.vector.tensor_scalar(
                    out=z[bt][:, mc * 512:(mc + 1) * 512], in0=zp[:],
                    scalar1=-1.0, scalar2=None, op0=ALU.add, op1=ALU.add,
                    accum_out=zsum[bt][:, mc:mc + 1])
                nc.vector.reduce_max(
                    mx[bt][:, mc:mc + 1], zp[:], axis=AX.X)
        ps_dist.release()

        # ---- sparsemax via log-secant; S(tau) = ACT head + DVE tail chunks ----
        ps_warm = tc.alloc_tile_pool(name="ps_warm", bufs=2, space="PSUM")
        CAP_OFF = 1e-4

        tail_scr = small.tile([P, 512], F32, tag="tailscr")

        def s_pass(bt, stt):
            tau_c, ntau = stt["tau_c"], stt["ntau"]
            s_act, gacc, s_v = stt["s_act"], stt["gacc"], stt["s_v"]
            nc.scalar.activation(
                w[bt][:, 0:MA], z[bt][:, 0:MA], AF.Relu,
                bias=ntau[:, 0:1], accum_out=s_act[:])
            for c in range(NDV):
                off = MA + c * 512
                # relu in two DVE ops: max into scratch, then (x - tau) with
                # fused add-reduce accum (sums small positives -> no cancellation)
                nc.vector.tensor_scalar(
                    out=tail_scr[:], in0=z[bt][:, off:off + 512],
                    scalar1=tau_c[:, 0:1], scalar2=None, op0=ALU.max)
                nc.vector.tensor_scalar(
                    out=w[bt][:, off:off + 512], in0=tail_scr[:],
                    scalar1=tau_c[:, 0:1], scalar2=None,
                    op0=ALU.subtract, op1=ALU.add, accum_out=gacc[:, c:c + 1])
            # S = s_act + sum(gacc)
            gs = stt["gs"]
            nc.vector.reduce_sum(gs[:], gacc[:], axis=AX.X)
            nc.vector.tensor_add(s_v[:], gs[:], s_act[:])
            # PE warmers: keep HAM at 8/8 through the sparsemax window
            wp = ps_warm.tile([P, 512], F32, tag="warm")
            nc.tensor.matmul(wp[:], ident[:], w[bt][:, 0:512],
                             start=True, stop=True)
            nc.tensor.matmul(wp[:], ident[:], w[bt][:, 512:1024],
                             start=True, stop=True)

        st = {}
        for bt in range(NBT):
            stt = {}
            for nm in ["rm", "cap", "tau_p", "tau_c", "l_p", "l_c", "ntau",
                       "s_v", "s_act", "gs", "corr", "dl", "rdl", "dt",
                       "step", "neg"]:
                stt[nm] = small.tile([P, 1], F32, tag=f"{nm}{bt}", name=f"{nm}{bt}")
            stt["gacc"] = small.tile([P, NDV], F32, tag=f"gacc{bt}", name=f"gacc{bt}")
            st[bt] = stt
            rm, cap, tau_p, tau_c = stt["rm"], stt["cap"], stt["tau_p"], stt["tau_c"]
            l_p, ntau, s_v = stt["l_p"], stt["ntau"], stt["s_v"]
            nc.vector.reduce_max(rm[:], mx[bt][:], axis=AX.X)
            nc.vector.tensor_scalar_add(rm[:], rm[:], -1.0)  # rowmax of z
            nc.vector.tensor_scalar_add(cap[:], rm[:], -CAP_OFF)
            nc.vector.tensor_scalar_add(tau_p[:], rm[:], -1.0)
            # analytic S0 = sum(z) - M*tau_p (tau_p = rowmax-1; <= true S, safe)
            zs = stt["gs"]
            nc.vector.reduce_sum(zs[:], zsum[bt][:], axis=AX.X)
            nc.vector.tensor_scalar_mul(s_v[:], tau_p[:], -float(M))
            nc.vector.tensor_add(s_v[:], s_v[:], zs[:])
            nc.vector.tensor_scalar_max(s_v[:], s_v[:], 1.0)  # guard ln<=0
            nc.scalar.activation(l_p[:], s_v[:], AF.Ln)
            # tau_c = tau_p + (S0-1)/M, capped
            nc.vector.tensor_scalar(
                out=tau_c[:], in0=s_v[:], scalar1=-1.0, scalar2=1.0 / M,
                op0=ALU.add, op1=ALU.mult)
            nc.vector.tensor_add(tau_c[:], tau_c[:], tau_p[:])
            nc.vector.tensor_tensor(tau_c[:], tau_c[:], cap[:], ALU.min)
            nc.vector.tensor_scalar_mul(ntau[:], tau_c[:], -1.0)
            s_pass(bt, stt)
            nc.scalar.activation(stt["l_c"][:], s_v[:], AF.Ln)

        for it in range(N_SECANT):
            for bt in range(NBT):
                stt = st[bt]
                cap, tau_p, tau_c = stt["cap"], stt["tau_p"], stt["tau_c"]
                l_p, l_c, ntau, s_v = stt["l_p"], stt["l_c"], stt["ntau"], stt["s_v"]
                dl, rdl, dt = stt["dl"], stt["rdl"], stt["dt"]
                step, neg = stt["step"], stt["neg"]
                nc.vector.tensor_sub(dl[:], l_p[:], l_c[:])
                nc.vector.tensor_scalar(
                    out=dl[:], in0=dl[:], scalar1=-1.0, scalar2=dl[:, 0:1],
                    op0=ALU.mult, op1=ALU.max)   # |dl|
                nc.vector.tensor_scalar_max(dl[:], dl[:], 1e-12)
                nc.vector.reciprocal(rdl[:], dl[:])
                nc.vector.tensor_sub(dt[:], tau_c[:], tau_p[:])
                nc.vector.tensor_scalar(
                    out=dt[:], in0=dt[:], scalar1=-1.0, scalar2=dt[:, 0:1],
                    op0=ALU.mult, op1=ALU.max)   # |dt|
                nc.vector.tensor_scalar(
                    out=step[:], in0=dt[:], scalar1=rdl[:, 0:1], scalar2=2.0,
                    op0=ALU.mult, op1=ALU.min)   # q = min(|dt|/|dl|, 2)
                nc.scalar.copy(tau_p[:], tau_c[:])
                nc.vector.tensor_mul(step[:], step[:], l_c[:])
                nc.vector.tensor_scalar(
                    out=tau_c[:], in0=step[:], scalar1=tau_c[:, 0:1],
                    scalar2=cap[:, 0:1], op0=ALU.add, op1=ALU.min)
                nc.scalar.copy(l_p[:], l_c[:])
                nc.vector.tensor_scalar_mul(ntau[:], tau_c[:], -1.0)
                s_pass(bt, stt)  # last iteration leaves w (head exact relu)
                if it != N_SECANT - 1:
                    nc.scalar.activation(l_c[:], s_v[:], AF.Ln)
        ps_warm.release()

        # ---- w^T transposes (wT reuses z0's slot) ----
        wTt = zpool.tile([P, MC128, B_CORE], BF16, tag="z0", name="wTt")
        ps_trw = tc.alloc_tile_pool(name="ps_trw", bufs=4, space="PSUM")
        for mc in range(MC128):
            tp = ps_trw.tile([P, B_CORE], F32, tag="wtr")
            for bt in range(NBT):
                nc.tensor.transpose(
                    tp[:, bt * P:(bt + 1) * P],
                    w[bt][:, mc * P:(mc + 1) * P], ident[:])
            nc.vector.tensor_copy(wTt[:, mc], tp[:])
        ps_trw.release()

        # W1 blocks prefetch into z1's slot (one contiguous DMA)
        w1t = zpool.tile([P, DC, HC, P], BF16, tag="z1", name="w1t")
        for dq in range(4):
            nc.sync.dma_start(w1t[:, dq * 2:(dq + 1) * 2],
                              w1_bf[:, dq * 2:(dq + 1) * 2])

        # ---- read matmul: lhsT = wT slice (few LDWEIGHTS), mv[bt][b, d] psum ----
        ps_mv = tc.alloc_tile_pool(name="ps_mv", bufs=1, space="PSUM")
        mv_ps = [[ps_mv.tile([P, 512], F32, tag=f"mv{bt}_{dh}", name=f"mv{bt}_{dh}")
                  for dh in range(2)] for bt in range(NBT)]
        for mc4 in range(MC128 // 4):
            mtile = mstream.tile([P, 4, D], BF16, tag="memT", name="membf")
            for dq in range(2):
                nc.sync.dma_start(
                    mtile[:, dq * 2:(dq + 1) * 2],
                    mem_bf[mc4 * 4 + dq * 2:mc4 * 4 + (dq + 1) * 2]
                    .rearrange("c p d -> p c d"))
            for c in range(4):
                mc = mc4 * 4 + c
                for bt in range(NBT):
                    for dh in range(2):
                        nc.tensor.matmul(
                            mv_ps[bt][dh][:],
                            wTt[:, mc, bt * P:(bt + 1) * P],
                            mtile[:, c, dh * 512:(dh + 1) * 512],
                            start=(mc == 0), stop=(mc == MC128 - 1))

        # evacuate mv to fp32 SBUF, transpose to mvT bf16 [P, dc, 256]
        mv_sb = [small.tile([P, D], F32, tag=f"mvsb{bt}", name=f"mvsb{bt}")
                 for bt in range(NBT)]
        for bt in range(NBT):
            for dh in range(2):
                nc.scalar.copy(mv_sb[bt][:, dh * 512:(dh + 1) * 512],
                               mv_ps[bt][dh][:])
        ps_mv.release()
        mvT = wpool.tile([P, DC, B_CORE], BF16, tag="w0", name="mvT")
        ps_mvt = tc.alloc_tile_pool(name="ps_mvt", bufs=4, space="PSUM")
        for dc in range(DC):
            tp = ps_mvt.tile([P, B_CORE], F32, tag="mvtr")
            for bt in range(NBT):
                nc.tensor.transpose(
                    tp[:, bt * P:(bt + 1) * P],
                    mv_sb[bt][:, dc * P:(dc + 1) * P], ident[:])
            nc.vector.tensor_copy(mvT[:, dc], tp[:])
        ps_mvt.release()

        # ---- MLP1: hT[hc] = relu(sum_dc W1-block^T @ mvT[dc] + b1[hc]) ----
        hT = wpool.tile([P, HC, B_CORE], BF16, tag="w1", name="hT")
        ps_h = tc.alloc_tile_pool(name="ps_h", bufs=4, space="PSUM")
        for hc in range(HC):
            hp = ps_h.tile([P, B_CORE], F32, tag="hp")
            for dc in range(DC):
                nc.tensor.matmul(
                    hp[:], w1t[:, dc, hc], mvT[:, dc],
                    start=(dc == 0), stop=(dc == DC - 1))
            nc.scalar.activation(
                hT[:, hc], hp[:], AF.Relu, bias=b1t[:, hc:hc + 1])
        ps_h.release()

        # ---- MLP2: out[bt] = hT-blocks^T @ W2 + b2 (nh outer, W2 slab DMA) ----
        ps_o = tc.alloc_tile_pool(name="ps_o", bufs=4, space="PSUM")
        osb = [small.tile([P, OUT], F32, tag=f"osb{bt}", name=f"osb{bt}")
               for bt in range(NBT)]
        NW = OUT // NH
        for nh in range(NH):
            w2slab = mstream.tile([P, HC, NW], BF16, tag="memT", name="w2slab")
            for dq in range(2):
                nc.sync.dma_start(
                    w2slab[:, dq * 8:(dq + 1) * 8],
                    w2_bf[:, dq * 8:(dq + 1) * 8, nh * NW:(nh + 1) * NW])
            ops = [ps_o.tile([P, NW], F32, tag=f"op{bt}", name=f"op{bt}")
                   for bt in range(NBT)]
            for kc in range(HC):
                for bt in range(NBT):
                    nc.tensor.matmul(
                        ops[bt][:], hT[:, kc, bt * P:(bt + 1) * P],
                        w2slab[:, kc], start=(kc == 0), stop=False)
            for bt in range(NBT):
                nc.tensor.matmul(
                    ops[bt][:], ones1[:], b2t[:, nh * NW:(nh + 1) * NW],
                    start=False, stop=True)
                nc.scalar.copy(osb[bt][:, nh * NW:(nh + 1) * NW], ops[bt][:])
        for bt in range(NBT):
            nc.sync.dma_start(out_d[bt], osb[bt][:])
        ps_o.release()

        mstream.release()
        zpool.release()
        wpool.release()
        pers.release()
        small.release()

    nc.compile()
    return nc


_CACHED = None


def _prep(inputs):
    x = np.ascontiguousarray(inputs["encoder_output"], dtype=np.float32)
    mem = np.ascontiguousarray(inputs["memory_set"], dtype=np.float32)
    W1 = np.ascontiguousarray(inputs["W1"], dtype=np.float32)
    b1 = np.ascontiguousarray(inputs["b1"], dtype=np.float32)
    W2 = np.ascontiguousarray(inputs["W2"], dtype=np.float32)
    b2 = np.ascontiguousarray(inputs["b2"], dtype=np.float32)

    inv_ny = 1.0 / np.sqrt((mem * mem).sum(1) + _EPS)
    memT_hat = np.ascontiguousarray(
        (mem.T * inv_ny[None, :]).astype(np.float32).reshape(DC, P, M))
    mem_bfv = np.ascontiguousarray(mem.astype(bf16).reshape(MC128, P, D))
    # partition-major blocks: w1[p, dc, hc, c] = W1[dc*128+p, hc*128+c]
    w1_blk = np.ascontiguousarray(
        W1.astype(bf16).reshape(DC, P, HC, P).transpose(1, 0, 2, 3))
    # w2[p, kc, o] = W2[kc*128+p, o]
    w2_blk = np.ascontiguousarray(
        W2.astype(bf16).reshape(HC, P, OUT).transpose(1, 0, 2))
    b1_tiles = np.ascontiguousarray(b1.reshape(HC, P).T.astype(np.float32))
    b2_row = np.ascontiguousarray(b2.reshape(1, OUT).astype(np.float32))

    shared = {
        "memT": memT_hat, "mem_bf": mem_bfv, "w1_bf": w1_blk,
        "w2_bf": w2_blk, "b1_t": b1_tiles, "b2_r": b2_row,
    }
    in_maps = []
    for c in range(8):
        xs = np.ascontiguousarray(
            x[c * B_CORE:(c + 1) * B_CORE].reshape(NBT, P, D))
        in_maps.append({"x_s": xs, **shared})
    return in_maps


def kernel(**inputs) -> np.ndarray:
    global _CACHED
    if _CACHED is None:
        _CACHED = build()
    nc = _CACHED
    in_maps = _prep(inputs)
    res = run_bass_kernel_spmd(nc, in_maps, core_ids=list(range(8)))
    return np.concatenate(
        [r["out"].reshape(B_CORE, OUT) for r in res.results], axis=0)
